# revision 5
# baseline (speedup 1.0000x reference)
"""Trainium2 Bass kernel: per-pixel channel shuffle + 3x3 conv (stride 1, pad 1).

Problem: x [32,256,56,56] f32, w [256,256,3,3] f32 (OIHW), perm [3136,256] i32;
out[b,:,h,w] = conv3x3(xs)[b,:,h,w] where xs[b,:,l] = x[b, perm[l,:], l].

Strategy (8 NeuronCores, data-parallel over batch, 4 batches/core):
  Shuffle (per batch, per 112-pixel tile): PE-transpose -> [l, c] PSUM, evict,
  GPSIMD local_scatter applies per-pixel inverse channel perms, PE-transpose
  back -> [c, l] into a zero-padded 58x58 flat image xs.
  Conv: Winograd F(2x2,3x3), with the row output-transform A^T folded into the
  transformed weights: Z[p,j] = sum_{i in I_p} sgn * (G w G^T)[i,j] (x) V[i,j]
  accumulates in PSUM over (i, ct); only the column transform (Y = combos of
  Z[p,j] over j) runs on DVE. Input transform V = (B^T d B) via two DVE
  tensor-op stages (T then V) on strided views of xs. 2.25x fewer PE cycles
  than direct conv.
"""

import os
import sys
import types
import numpy as np

_STATE = {}
LAST_RESULT = None

B, C, H, W = 32, 256, 56, 56
HW = H * W
PADW = 58
XS_LEN = 3440          # padded 58x58 image (3364) + slack for strided views
TL = 112
NT = 28
N_CORES = 8
B_LOC = B // N_CORES
TT = 28 * 28           # winograd tiles per image (28x28 of 2x2 outputs)
NG = 392               # matmul N per (ct-block, ty-half)

# U'-tile variants: (i, sign) pairs actually needed by Z[p, j] accumulation
VARIANTS = [(0, 1), (1, 1), (2, 1), (2, -1), (3, -1)]
VIDX = {v: k for k, v in enumerate(VARIANTS)}
IP = {0: [(0, 1), (1, 1), (2, 1)], 1: [(1, 1), (2, -1), (3, -1)]}


def _tid(v, j, ct, oc):
    return ((v * 4 + j) * 2 + ct) * 2 + oc


def _install_ntff_shim():
    name = "antenv.axon_hooks"
    if name in sys.modules:
        return
    try:
        import antenv  # noqa: F401

        m = types.ModuleType(name)
        m._hook = None
        m.set_axon_ntff_profile_hook = lambda h: setattr(m, "_hook", h)
        m.get_axon_ntff_profile_hook = lambda: m._hook
        sys.modules[name] = m
        setattr(sys.modules["antenv"], "axon_hooks", m)
        from trn_agent_boot.trn_boot import _ntff_profile_via_ctypes

        hook = _ntff_profile_via_ctypes("/opt/axon/libaxon_pjrt.so")
        if hook is not None:
            m.set_axon_ntff_profile_hook(hook)
    except Exception:
        pass


def _build_kernel():
    import concourse.bass as bass
    import concourse.mybir as mybir
    from concourse import bacc, tile
    from concourse.masks import make_identity
    from contextlib import ExitStack

    F32 = mybir.dt.float32
    BF16 = mybir.dt.bfloat16
    I16 = mybir.dt.int16

    nc = bacc.Bacc("TRN2", target_bir_lowering=False, debug=False, num_devices=N_CORES)

    xb = nc.dram_tensor("xb", [B_LOC, C, HW], BF16, kind="ExternalInput")
    up = nc.dram_tensor("up", [80, 128, 128], BF16, kind="ExternalInput")
    idxt = nc.dram_tensor("idxt", [128, NT * 256], I16, kind="ExternalInput")
    out = nc.dram_tensor("out", [B_LOC, C, HW], F32, kind="ExternalOutput")

    with tile.TileContext(nc) as tc, ExitStack() as ctx:
        const = ctx.enter_context(tc.tile_pool(name="const", bufs=1))
        upsb = const.tile([128, 80 * 128], BF16)
        nc.sync.dma_start(
            out=upsb[:, :],
            in_=bass.AP(up, 0, [[128, 128], [128 * 128, 80], [1, 128]]),
        )
        idxsb = const.tile([128, NT * 256], I16)
        nc.sync.dma_start(out=idxsb[:, :], in_=idxt[:, :])
        ident = const.tile([128, 128], BF16)
        make_identity(nc, ident[:, :])

        xin_pool = ctx.enter_context(tc.tile_pool(name="xin", bufs=2))
        xs_pool = ctx.enter_context(tc.tile_pool(name="xs", bufs=2))
        gin_pool = ctx.enter_context(tc.tile_pool(name="gin", bufs=4))
        sout_pool = ctx.enter_context(tc.tile_pool(name="sout", bufs=4))
        t_pool = ctx.enter_context(tc.tile_pool(name="tp", bufs=6))
        v_pool = ctx.enter_context(tc.tile_pool(name="vp", bufs=17))
        tt_pool = ctx.enter_context(tc.tile_pool(name="tt", bufs=4))
        y_pool = ctx.enter_context(tc.tile_pool(name="yb", bufs=2))
        tpsA_pool = ctx.enter_context(tc.tile_pool(name="tpsA", bufs=2, space="PSUM"))
        tpsB_pool = ctx.enter_context(tc.tile_pool(name="tpsB", bufs=2, space="PSUM"))
        z_pool = ctx.enter_context(tc.tile_pool(name="zp", bufs=4, space="PSUM"))

        for b in range(B_LOC):
            # ---------------- phase 1: channel shuffle ----------------
            xin = xin_pool.tile([128, 2 * HW], BF16)
            for ct in range(2):
                nc.sync.dma_start(
                    out=xin[:, ct * HW : (ct + 1) * HW],
                    in_=xb[b, ct * 128 : (ct + 1) * 128, :],
                )

            xs = xs_pool.tile([128, 2 * XS_LEN], BF16)
            for ct in range(2):
                base = ct * XS_LEN
                nc.vector.memset(xs[:, base : base + PADW], 0.0)
                nc.vector.memset(
                    xs[:, base + 57 * PADW : base + 58 * PADW], 0.0
                )
                nc.vector.memset(
                    xs[:, base + PADW : base + PADW + 56 * PADW].rearrange(
                        "p (r x) -> p r x", r=56
                    )[:, :, 0:1],
                    0.0,
                )
                nc.vector.memset(
                    xs[:, base + PADW + 57 : base + PADW + 57 + 56 * PADW].rearrange(
                        "p (r x) -> p r x", r=56
                    )[:, :, 0:1],
                    0.0,
                )

            for t in range(NT):
                ps1 = tpsA_pool.tile([128, 256], BF16, name="ps1", tag="ps1")
                for ct in range(2):
                    nc.tensor.transpose(
                        ps1[0:TL, ct * 128 : ct * 128 + 128],
                        xin[:, ct * HW + t * TL : ct * HW + t * TL + TL],
                        ident[:, :],
                    )
                gin = gin_pool.tile([128, 256], BF16, name="gin", tag="gin")
                if t % 2 == 0:
                    nc.scalar.copy(gin[0:TL, :], ps1[0:TL, :])
                else:
                    nc.vector.tensor_copy(gin[0:TL, :], ps1[0:TL, :])
                sout = sout_pool.tile([128, 256], BF16, name="sout", tag="sout")
                nc.gpsimd.local_scatter(
                    out_ap=sout[0:TL, :],
                    data_ap=gin[0:TL, :],
                    idxs_ap=idxsb[0:TL, t * 256 : (t + 1) * 256],
                    channels=TL,
                    num_elems=256,
                    num_idxs=256,
                )
                ps2 = tpsB_pool.tile([128, 2 * TL], BF16, name="ps2", tag="ps2")
                for ct in range(2):
                    nc.tensor.transpose(
                        ps2[:, ct * TL : ct * TL + TL],
                        sout[0:TL, ct * 128 : ct * 128 + 128],
                        ident[0:TL, 0:TL],
                    )
                q = 59 + 2 * t * PADW
                for ct in range(2):
                    nc.vector.tensor_copy(
                        xs[:, ct * XS_LEN + q : ct * XS_LEN + q + 2 * PADW].rearrange(
                            "p (r x) -> p r x", r=2
                        )[:, :, 0:56],
                        ps2[:, ct * TL : ct * TL + TL].rearrange(
                            "p (r x) -> p r x", r=2
                        ),
                    )

            # ---------------- phase 2: Winograd conv ----------------
            # strided view of xs: d(a, bp)[p, ct, ty, tx] = xs[p, ct, (2ty+a)*58 + 2tx+bp]
            def dview(a, bp):
                off = a * PADW + bp
                v = xs[:, :].rearrange("p (ct q) -> p ct q", ct=2)
                v = v[:, :, off : off + 28 * 116]
                v = v.rearrange("p ct (ty u) -> p ct ty u", u=116)
                v = v.rearrange("p ct ty (vv ww) -> p ct ty vv ww", ww=2)
                return v[:, :, :, 0:28, 0:1]

            # T stage (i-blocked), then V stage
            vt = {}
            for i in range(4):
                tt = {}
                for bp in range(4):
                    tti = t_pool.tile([128, 2 * TT], BF16, name="tti")
                    tv = tti[:, :].rearrange(
                        "p (ct ty tx q) -> p ct ty tx q", ct=2, tx=28, q=1
                    )
                    if i == 0:
                        nc.vector.tensor_sub(tv, dview(0, bp), dview(2, bp))
                    elif i == 1:
                        nc.vector.tensor_add(tv, dview(1, bp), dview(2, bp))
                    elif i == 2:
                        nc.vector.tensor_sub(tv, dview(2, bp), dview(1, bp))
                    else:
                        nc.vector.tensor_sub(tv, dview(1, bp), dview(3, bp))
                    tt[bp] = tti
                for j in range(4):
                    vij = v_pool.tile([128, 2 * TT], BF16, name="vij")
                    if j == 0:
                        nc.vector.tensor_sub(vij[:, :], tt[0][:, :], tt[2][:, :])
                    elif j == 1:
                        nc.vector.tensor_add(vij[:, :], tt[1][:, :], tt[2][:, :])
                    elif j == 2:
                        nc.vector.tensor_sub(vij[:, :], tt[2][:, :], tt[1][:, :])
                    else:
                        nc.vector.tensor_sub(vij[:, :], tt[1][:, :], tt[3][:, :])
                    vt[(i, j)] = vij

            for half in range(2):
                for oc in range(2):
                    ybuf = y_pool.tile([128, 1568], F32, name="ybuf")
                    yv = ybuf[:, :].rearrange(
                        "p (t r c q) -> p t r c q", t=14, r=2, q=2
                    )
                    for p in range(2):
                        zt = {}
                        for stage, js in enumerate(((1, 2), (0, 3))):
                            for j in js:
                                z = z_pool.tile([128, NG], F32, name="z")
                                k = 0
                                for (i, sgn) in IP[p]:
                                    v = VIDX[(i, sgn)]
                                    for ct in range(2):
                                        nc.tensor.matmul(
                                            z[:, :],
                                            lhsT=upsb[
                                                :,
                                                _tid(v, j, ct, oc) * 128 : (
                                                    _tid(v, j, ct, oc) + 1
                                                )
                                                * 128,
                                            ],
                                            rhs=vt[(i, j)][
                                                :,
                                                ct * TT
                                                + half * NG : ct * TT
                                                + half * NG
                                                + NG,
                                            ],
                                            start=(k == 0),
                                            stop=(k == 5),
                                        )
                                        k += 1
                                zt[j] = z
                            if stage == 0:
                                zc = tt_pool.tile([128, NG], F32, name="zc")
                                nc.scalar.copy(zc[:, :], zt[1][:, :])
                                t0 = tt_pool.tile([128, NG], F32, name="t0")
                                t1 = tt_pool.tile([128, NG], F32, name="t0")
                                nc.vector.tensor_add(t0[:, :], zc[:, :], zt[2][:, :])
                                nc.vector.tensor_sub(t1[:, :], zc[:, :], zt[2][:, :])
                        y0 = yv[:, :, p : p + 1, :, 0:1]
                        y1 = yv[:, :, p : p + 1, :, 1:2]
                        tshape = lambda ap: ap.rearrange(
                            "p (t r c q) -> p t r c q", t=14, r=1, c=28, q=1
                        )
                        nc.vector.tensor_add(y0, tshape(t0[:, :]), tshape(zt[0][:, :]))
                        nc.vector.tensor_sub(y1, tshape(t1[:, :]), tshape(zt[3][:, :]))
                    nc.sync.dma_start(
                        out=out[
                            b,
                            oc * 128 : (oc + 1) * 128,
                            half * 1568 : (half + 1) * 1568,
                        ],
                        in_=ybuf[:, :],
                    )

    nc.compile()
    return nc


def _host_prep(x, w, perm):
    import ml_dtypes

    xf = x.reshape(B, C, HW).astype(ml_dtypes.bfloat16)

    # winograd-transformed weights with folded A^T row transform signs
    G = np.array([[1, 0, 0], [0.5, 0.5, 0.5], [0.5, -0.5, 0.5], [0, 0, 1]], np.float32)
    U = np.einsum("ia,ocab,jb->ijoc", G, w.astype(np.float32), G)  # [4,4,OC,C]
    upt = np.empty((80, 128, 128), dtype=ml_dtypes.bfloat16)
    for (i, s) in VARIANTS:
        v = VIDX[(i, s)]
        for j in range(4):
            for ct in range(2):
                for oc in range(2):
                    blk = s * U[i, j][oc * 128 : (oc + 1) * 128, ct * 128 : (ct + 1) * 128]
                    upt[_tid(v, j, ct, oc)] = blk.T.astype(ml_dtypes.bfloat16)

    iperm = np.empty((HW, C), dtype=np.int16)
    np.put_along_axis(
        iperm, perm.astype(np.int64), np.arange(C, dtype=np.int16)[None, :], axis=1
    )
    idxt = np.zeros((128, NT * 256), dtype=np.int16)
    for t in range(NT):
        idxt[0:TL, t * 256 : (t + 1) * 256] = iperm[t * TL : t * TL + TL, :]

    in_maps = []
    for cidx in range(N_CORES):
        in_maps.append(
            {
                "xb": np.ascontiguousarray(xf[cidx * B_LOC : (cidx + 1) * B_LOC]),
                "up": upt,
                "idxt": idxt,
            }
        )
    return in_maps


def kernel(x, w, perm):
    global LAST_RESULT
    _install_ntff_shim()
    from concourse.bass_utils import run_bass_kernel_spmd

    x = np.asarray(x, dtype=np.float32)
    w = np.asarray(w, dtype=np.float32)
    perm = np.asarray(perm)

    if "nc" not in _STATE:
        _STATE["nc"] = _build_kernel()
    nc = _STATE["nc"]

    in_maps = _host_prep(x, w, perm)
    res = run_bass_kernel_spmd(nc, in_maps, core_ids=list(range(N_CORES)))
    LAST_RESULT = res
    out = np.concatenate(
        [r["out"].reshape(B_LOC, C, H, W) for r in res.results], axis=0
    )
    return out.astype(np.float32)


# revision 6
# speedup vs baseline: 1.4369x; 1.4369x over previous
"""Trainium2 Bass kernel: per-pixel channel shuffle + 3x3 conv (stride 1, pad 1).

Problem: x [32,256,56,56] f32, w [256,256,3,3] f32 (OIHW), perm [3136,256] i32;
out[b,:,h,w] = conv3x3(xs)[b,:,h,w] where xs[b,:,l] = x[b, perm[l,:], l].

Strategy (8 NeuronCores, data-parallel over batch, 4 batches/core):
  Shuffle (per batch, per 112-pixel tile): host pre-transposes x to [l, c]
  layout, DMA tiles straight into SBUF, GPSIMD local_scatter applies per-pixel
  inverse channel perms, PE-transpose back -> [c, l] into a zero-padded 58x58
  flat image xs.
  Conv: 1-D Winograd F(2,3) along x (1.5x fewer PE MACs than direct, at 1/8
  the DVE transform cost of the 2-D variant). V[j] = B^T-combos of stride-2
  column slices of xs (4 big DVE ops per batch); M[j] accumulates 3 kh-taps x
  2 ic-tiles in PSUM; the column output transform Y = A^T-combos of M[j] runs
  on DVE/Scalar reading PSUM, writing interleaved output column pairs.
"""

import os
import sys
import types
import numpy as np

_STATE = {}
LAST_RESULT = None

B, C, H, W = 32, 256, 56, 56
HW = H * W
PADW = 58
XS_LEN = 3440          # padded 58x58 image (3364) + slack for strided views
TL = 112
NT = 28
N_CORES = 8
B_LOC = B // N_CORES
TX = 28                # winograd column tiles (pairs of output columns)
VL = PADW * TX         # V[j] length per ct: 58 rows x 28 tiles
NG = 392               # matmul N: 14 output rows x 28 tiles
NGRP = 4               # row groups of 14


def _tid1(j, kh, ct, oc):
    return ((j * 3 + kh) * 2 + ct) * 2 + oc


def _install_ntff_shim():
    name = "antenv.axon_hooks"
    if name in sys.modules:
        return
    try:
        import antenv  # noqa: F401

        m = types.ModuleType(name)
        m._hook = None
        m.set_axon_ntff_profile_hook = lambda h: setattr(m, "_hook", h)
        m.get_axon_ntff_profile_hook = lambda: m._hook
        sys.modules[name] = m
        setattr(sys.modules["antenv"], "axon_hooks", m)
        from trn_agent_boot.trn_boot import _ntff_profile_via_ctypes

        hook = _ntff_profile_via_ctypes("/opt/axon/libaxon_pjrt.so")
        if hook is not None:
            m.set_axon_ntff_profile_hook(hook)
    except Exception:
        pass


def _build_kernel():
    import concourse.bass as bass
    import concourse.mybir as mybir
    from concourse import bacc, tile
    from concourse.masks import make_identity
    from contextlib import ExitStack

    F32 = mybir.dt.float32
    BF16 = mybir.dt.bfloat16
    I16 = mybir.dt.int16

    nc = bacc.Bacc("TRN2", target_bir_lowering=False, debug=False, num_devices=N_CORES)

    xbt = nc.dram_tensor("xbt", [B_LOC, HW, C], BF16, kind="ExternalInput")
    u1 = nc.dram_tensor("u1", [48, 128, 128], BF16, kind="ExternalInput")
    idxt = nc.dram_tensor("idxt", [128, NT * 256], I16, kind="ExternalInput")
    out = nc.dram_tensor("out", [B_LOC, C, HW], F32, kind="ExternalOutput")

    with tile.TileContext(nc) as tc, ExitStack() as ctx:
        const = ctx.enter_context(tc.tile_pool(name="const", bufs=1))
        usb = const.tile([128, 48 * 128], BF16)
        nc.sync.dma_start(
            out=usb[:, :],
            in_=bass.AP(u1, 0, [[128, 128], [128 * 128, 48], [1, 128]]),
        )
        idxsb = const.tile([128, NT * 256], I16)
        nc.sync.dma_start(out=idxsb[:, :], in_=idxt[:, :])
        ident = const.tile([128, 128], BF16)
        make_identity(nc, ident[:, :])

        xs_pool = ctx.enter_context(tc.tile_pool(name="xs", bufs=2))
        gin_pool = ctx.enter_context(tc.tile_pool(name="gin", bufs=6))
        sout_pool = ctx.enter_context(tc.tile_pool(name="sout", bufs=6))
        v_pool = ctx.enter_context(tc.tile_pool(name="vp", bufs=8))
        tt_pool = ctx.enter_context(tc.tile_pool(name="tt", bufs=4))
        y_pool = ctx.enter_context(tc.tile_pool(name="yb", bufs=4))
        tpsB_pool = ctx.enter_context(tc.tile_pool(name="tpsB", bufs=3, space="PSUM"))
        z_pool = ctx.enter_context(tc.tile_pool(name="zp", bufs=4, space="PSUM"))

        for b in range(B_LOC):
            # ---------------- phase 1: channel shuffle ----------------
            xs = xs_pool.tile([128, 2 * XS_LEN], BF16)
            for ct in range(2):
                base = ct * XS_LEN
                nc.vector.memset(xs[:, base : base + PADW], 0.0)
                nc.vector.memset(xs[:, base + 57 * PADW : base + 58 * PADW], 0.0)
                nc.vector.memset(
                    xs[:, base + PADW : base + PADW + 56 * PADW].rearrange(
                        "p (r x) -> p r x", r=56
                    )[:, :, 0:1],
                    0.0,
                )
                nc.vector.memset(
                    xs[:, base + PADW + 57 : base + PADW + 57 + 56 * PADW].rearrange(
                        "p (r x) -> p r x", r=56
                    )[:, :, 0:1],
                    0.0,
                )

            for t in range(NT):
                gin = gin_pool.tile([128, 256], BF16, name="gin", tag="gin")
                nc.sync.dma_start(
                    out=gin[0:TL, :], in_=xbt[b, t * TL : (t + 1) * TL, :]
                )
                sout = sout_pool.tile([128, 256], BF16, name="sout", tag="sout")
                nc.gpsimd.local_scatter(
                    out_ap=sout[0:TL, :],
                    data_ap=gin[0:TL, :],
                    idxs_ap=idxsb[0:TL, t * 256 : (t + 1) * 256],
                    channels=TL,
                    num_elems=256,
                    num_idxs=256,
                )
                ps2 = tpsB_pool.tile([128, 2 * TL], BF16, name="ps2", tag="ps2")
                for ct in range(2):
                    nc.tensor.transpose(
                        ps2[:, ct * TL : ct * TL + TL],
                        sout[0:TL, ct * 128 : ct * 128 + 128],
                        ident[0:TL, 0:TL],
                    )
                q = 59 + 2 * t * PADW
                for ct in range(2):
                    dst = xs[
                        :, ct * XS_LEN + q : ct * XS_LEN + q + 2 * PADW
                    ].rearrange("p (r x) -> p r x", r=2)[:, :, 0:56]
                    src = ps2[:, ct * TL : ct * TL + TL].rearrange(
                        "p (r x) -> p r x", r=2
                    )
                    if ct == 0:
                        nc.scalar.copy(dst, src)
                    else:
                        nc.vector.tensor_copy(dst, src)

            # ---------------- phase 2: 1-D Winograd F(2,3) ----------------
            # d_b view: [p, ct, yy, tx] = xs[p, ct, yy*58 + 2*tx + b_]
            def dview(b_):
                v = xs[:, :].rearrange("p (ct q) -> p ct q", ct=2)
                v = v[:, :, b_ : b_ + PADW * PADW]
                v = v.rearrange("p ct (yy u) -> p ct yy u", u=PADW)
                v = v.rearrange("p ct yy (vv ww) -> p ct yy vv ww", ww=2)
                return v[:, :, :, 0:TX, 0:1]

            vt = []
            for j in range(4):
                vj = v_pool.tile([128, 2 * VL], BF16, name="vj")
                vv = vj[:, :].rearrange(
                    "p (ct yy tx q) -> p ct yy tx q", ct=2, tx=TX, q=1
                )
                if j == 0:
                    nc.vector.tensor_sub(vv, dview(0), dview(2))
                elif j == 1:
                    nc.vector.tensor_add(vv, dview(1), dview(2))
                elif j == 2:
                    nc.vector.tensor_sub(vv, dview(2), dview(1))
                else:
                    nc.vector.tensor_sub(vv, dview(1), dview(3))
                vt.append(vj)

            for grp in range(NGRP):
                for oc in range(2):
                    ybuf = y_pool.tile([128, 14 * 56], F32, name="ybuf")
                    yv = ybuf[:, :].rearrange("p (y c q) -> p y c q", y=14, q=2)
                    zt = {}
                    for stage, js in enumerate(((1, 2), (0, 3))):
                        for j in js:
                            z = z_pool.tile([128, NG], F32, name="z")
                            k = 0
                            for kh in range(3):
                                for ct in range(2):
                                    nc.tensor.matmul(
                                        z[:, :],
                                        lhsT=usb[
                                            :,
                                            _tid1(j, kh, ct, oc) * 128 : (
                                                _tid1(j, kh, ct, oc) + 1
                                            )
                                            * 128,
                                        ],
                                        rhs=vt[j][
                                            :,
                                            ct * VL
                                            + (grp * 14 + kh) * TX : ct * VL
                                            + (grp * 14 + kh) * TX
                                            + NG,
                                        ],
                                        start=(k == 0),
                                        stop=(k == 5),
                                    )
                                    k += 1
                            zt[j] = z
                        if stage == 0:
                            zc = tt_pool.tile([128, NG], F32, name="zc")
                            nc.scalar.copy(zc[:, :], zt[1][:, :])
                            t0 = tt_pool.tile([128, NG], F32, name="t0")
                            t1 = tt_pool.tile([128, NG], F32, name="t0")
                            nc.vector.tensor_add(t0[:, :], zc[:, :], zt[2][:, :])
                            nc.vector.tensor_sub(t1[:, :], zc[:, :], zt[2][:, :])
                    tshape = lambda ap: ap.rearrange(
                        "p (y c q) -> p y c q", y=14, c=28, q=1
                    )
                    nc.vector.tensor_add(
                        yv[:, :, :, 0:1], tshape(t0[:, :]), tshape(zt[0][:, :])
                    )
                    nc.vector.tensor_sub(
                        yv[:, :, :, 1:2], tshape(t1[:, :]), tshape(zt[3][:, :])
                    )
                    nc.sync.dma_start(
                        out=out[
                            b,
                            oc * 128 : (oc + 1) * 128,
                            grp * 14 * 56 : (grp + 1) * 14 * 56,
                        ],
                        in_=ybuf[:, :],
                    )

    nc.compile()
    return nc


def _host_prep(x, w, perm):
    import ml_dtypes

    # [B, HW, C] pixel-major bf16 (feeds the scatter without PE fwd transposes)
    xft = np.ascontiguousarray(
        x.reshape(B, C, HW).transpose(0, 2, 1)
    ).astype(ml_dtypes.bfloat16)

    # 1-D winograd weights: U1[j,kh][oc,ic] = sum_kw G[j,kw] w[oc,ic,kh,kw]
    G = np.array([[1, 0, 0], [0.5, 0.5, 0.5], [0.5, -0.5, 0.5], [0, 0, 1]], np.float32)
    U1 = np.einsum("jk,ochk->jhoc", G, w.astype(np.float32))  # [4,3,OC,C]
    u1t = np.empty((48, 128, 128), dtype=ml_dtypes.bfloat16)
    for j in range(4):
        for kh in range(3):
            for ct in range(2):
                for oc in range(2):
                    blk = U1[j, kh][oc * 128 : (oc + 1) * 128, ct * 128 : (ct + 1) * 128]
                    u1t[_tid1(j, kh, ct, oc)] = blk.T.astype(ml_dtypes.bfloat16)

    iperm = np.empty((HW, C), dtype=np.int16)
    np.put_along_axis(
        iperm, perm.astype(np.int64), np.arange(C, dtype=np.int16)[None, :], axis=1
    )
    idxt = np.zeros((128, NT * 256), dtype=np.int16)
    for t in range(NT):
        idxt[0:TL, t * 256 : (t + 1) * 256] = iperm[t * TL : t * TL + TL, :]

    in_maps = []
    for cidx in range(N_CORES):
        in_maps.append(
            {
                "xbt": np.ascontiguousarray(xft[cidx * B_LOC : (cidx + 1) * B_LOC]),
                "u1": u1t,
                "idxt": idxt,
            }
        )
    return in_maps


def kernel(x, w, perm):
    global LAST_RESULT
    _install_ntff_shim()
    from concourse.bass_utils import run_bass_kernel_spmd

    x = np.asarray(x, dtype=np.float32)
    w = np.asarray(w, dtype=np.float32)
    perm = np.asarray(perm)

    if "nc" not in _STATE:
        _STATE["nc"] = _build_kernel()
    nc = _STATE["nc"]

    in_maps = _host_prep(x, w, perm)
    res = run_bass_kernel_spmd(nc, in_maps, core_ids=list(range(N_CORES)))
    LAST_RESULT = res
    out = np.concatenate(
        [r["out"].reshape(B_LOC, C, H, W) for r in res.results], axis=0
    )
    return out.astype(np.float32)


# revision 7
# speedup vs baseline: 1.6777x; 1.1675x over previous
"""Trainium2 Bass kernel: per-pixel channel shuffle + 3x3 conv (stride 1, pad 1).

Problem: x [32,256,56,56] f32, w [256,256,3,3] f32 (OIHW), perm [3136,256] i32;
out[b,:,h,w] = conv3x3(xs)[b,:,h,w] where xs[b,:,l] = x[b, perm[l,:], l].

Strategy (8 NeuronCores, data-parallel over batch, 4 batches/core):
  Shuffle (per batch-pair, per 112-pixel tile): host pre-transposes x to
  [l, c] layout, DMA tiles straight into SBUF, one GPSIMD local_scatter per
  tile applies per-pixel inverse channel perms for BOTH batches of the pair
  (512 idxs), PE-transpose back -> [c, l] into a zero-padded 58x58 image xs
  stored row-interleaved ([row][ct][col]) so row-band consumers get precise
  sub-tile deps.
  Conv: 1-D Winograd F(2,3) along x (1.5x fewer PE MACs than direct). V[j]
  computed per 14-row group (fine-grained pipeline with the shuffle); M[j]
  accumulates 3 kh-taps x 2 ic-tiles in PSUM; output transform Y = A^T-combos
  of M[j] on DVE/Scalar, writing interleaved output column pairs.
"""

import os
import sys
import types
import numpy as np

_STATE = {}
LAST_RESULT = None

B, C, H, W = 32, 256, 56, 56
HW = H * W
PADW = 58
XROW = 2 * PADW        # one padded row holds both ct halves
XS_TOT = 58 * XROW
TL = 112
NT = 28
N_CORES = 8
B_LOC = B // N_CORES
TX = 28                # winograd column tiles (pairs of output columns)
NG = 392               # matmul N: 14 output rows x 28 tiles
NGRP = 4               # row groups of 14
VCH = 16 * TX          # V chunk per ct: 16 rows x 28 tiles


def _tid1(j, kh, ct, oc):
    return ((j * 3 + kh) * 2 + ct) * 2 + oc


def _install_ntff_shim():
    name = "antenv.axon_hooks"
    if name in sys.modules:
        return
    try:
        import antenv  # noqa: F401

        m = types.ModuleType(name)
        m._hook = None
        m.set_axon_ntff_profile_hook = lambda h: setattr(m, "_hook", h)
        m.get_axon_ntff_profile_hook = lambda: m._hook
        sys.modules[name] = m
        setattr(sys.modules["antenv"], "axon_hooks", m)
        from trn_agent_boot.trn_boot import _ntff_profile_via_ctypes

        hook = _ntff_profile_via_ctypes("/opt/axon/libaxon_pjrt.so")
        if hook is not None:
            m.set_axon_ntff_profile_hook(hook)
    except Exception:
        pass


def _build_kernel():
    import concourse.bass as bass
    import concourse.mybir as mybir
    from concourse import bacc, tile
    from concourse.masks import make_identity
    from contextlib import ExitStack

    F32 = mybir.dt.float32
    BF16 = mybir.dt.bfloat16
    I16 = mybir.dt.int16

    nc = bacc.Bacc("TRN2", target_bir_lowering=False, debug=False, num_devices=N_CORES)

    xbt = nc.dram_tensor("xbt", [B_LOC, HW, C], BF16, kind="ExternalInput")
    u1 = nc.dram_tensor("u1", [48, 128, 128], BF16, kind="ExternalInput")
    idxt = nc.dram_tensor("idxt", [128, NT * 512], I16, kind="ExternalInput")
    out = nc.dram_tensor("out", [B_LOC, C, HW], F32, kind="ExternalOutput")

    with tile.TileContext(nc) as tc, ExitStack() as ctx:
        const = ctx.enter_context(tc.tile_pool(name="const", bufs=1))
        usb = const.tile([128, 48 * 128], BF16)
        nc.sync.dma_start(
            out=usb[:, :],
            in_=bass.AP(u1, 0, [[128, 128], [128 * 128, 48], [1, 128]]),
        )
        idxsb = const.tile([128, NT * 512], I16)
        nc.sync.dma_start(out=idxsb[:, :], in_=idxt[:, :])
        ident = const.tile([128, 128], BF16)
        make_identity(nc, ident[:, :])

        xs_pool = ctx.enter_context(tc.tile_pool(name="xs", bufs=4))
        gin_pool = ctx.enter_context(tc.tile_pool(name="gin", bufs=5))
        sout_pool = ctx.enter_context(tc.tile_pool(name="sout", bufs=5))
        v_pool = ctx.enter_context(tc.tile_pool(name="vp", bufs=12))
        tt_pool = ctx.enter_context(tc.tile_pool(name="tt", bufs=4))
        y_pool = ctx.enter_context(tc.tile_pool(name="yb", bufs=4))
        tpsB_pool = ctx.enter_context(tc.tile_pool(name="tpsB", bufs=3, space="PSUM"))
        z_pool = ctx.enter_context(tc.tile_pool(name="zp", bufs=4, space="PSUM"))

        def phase2(b, xs):
            # d_b view for row group g: [p, yy, ct, tx, 1] =
            #   xs[p, (g*14+yy)*116 + ct*58 + 2*tx + b_]
            def dview(b_, g):
                v = xs[:, :].rearrange("p (yy u) -> p yy u", u=XROW)
                v = v[:, g * 14 : g * 14 + 16, :]
                v = v.rearrange("p yy (ct u2) -> p yy ct u2", ct=2)
                v = v.rearrange("p yy ct (vv ww) -> p yy ct vv ww", ww=2)
                return v[:, :, :, b_ // 2 : b_ // 2 + TX, b_ % 2 : b_ % 2 + 1]

            for g in range(NGRP):
                vt = []
                for j in range(4):
                    vj = v_pool.tile([128, 2 * VCH], BF16, name="vj")
                    vv = vj[:, :].rearrange(
                        "p (ct yy tx q) -> p yy ct tx q", ct=2, tx=TX, q=1
                    )
                    if j == 0:
                        nc.vector.tensor_sub(vv, dview(0, g), dview(2, g))
                    elif j == 1:
                        nc.vector.tensor_add(vv, dview(1, g), dview(2, g))
                    elif j == 2:
                        nc.vector.tensor_sub(vv, dview(2, g), dview(1, g))
                    else:
                        nc.vector.tensor_sub(vv, dview(1, g), dview(3, g))
                    vt.append(vj)

                for oc in range(2):
                    ybuf = y_pool.tile([128, 14 * 56], F32, name="ybuf")
                    yv = ybuf[:, :].rearrange("p (y c q) -> p y c q", y=14, q=2)
                    zt = {}
                    for stage, js in enumerate(((1, 2), (0, 3))):
                        for j in js:
                            z = z_pool.tile([128, NG], F32, name="z")
                            k = 0
                            for kh in range(3):
                                for ct in range(2):
                                    nc.tensor.matmul(
                                        z[:, :],
                                        lhsT=usb[
                                            :,
                                            _tid1(j, kh, ct, oc) * 128 : (
                                                _tid1(j, kh, ct, oc) + 1
                                            )
                                            * 128,
                                        ],
                                        rhs=vt[j][
                                            :,
                                            ct * VCH
                                            + kh * TX : ct * VCH
                                            + kh * TX
                                            + NG,
                                        ],
                                        start=(k == 0),
                                        stop=(k == 5),
                                    )
                                    k += 1
                            zt[j] = z
                        if stage == 0:
                            zc = tt_pool.tile([128, NG], F32, name="zc")
                            nc.scalar.copy(zc[:, :], zt[1][:, :])
                            t0 = tt_pool.tile([128, NG], F32, name="t0")
                            t1 = tt_pool.tile([128, NG], F32, name="t0")
                            nc.vector.tensor_add(t0[:, :], zc[:, :], zt[2][:, :])
                            nc.vector.tensor_sub(t1[:, :], zc[:, :], zt[2][:, :])
                    tshape = lambda ap: ap.rearrange(
                        "p (y c q) -> p y c q", y=14, c=28, q=1
                    )
                    nc.vector.tensor_add(
                        yv[:, :, :, 0:1], tshape(t0[:, :]), tshape(zt[0][:, :])
                    )
                    nc.vector.tensor_sub(
                        yv[:, :, :, 1:2], tshape(t1[:, :]), tshape(zt[3][:, :])
                    )
                    nc.sync.dma_start(
                        out=out[
                            b,
                            oc * 128 : (oc + 1) * 128,
                            g * 14 * 56 : (g + 1) * 14 * 56,
                        ],
                        in_=ybuf[:, :],
                    )

        for pair in range(B_LOC // 2):
            b0, b1 = 2 * pair, 2 * pair + 1
            xst = {}
            for b in (b0, b1):
                xs = xs_pool.tile([128, XS_TOT], BF16)
                xst[b] = xs
                xv = xs[:, :].rearrange("p (yy u) -> p yy u", u=XROW)
                nc.vector.memset(xs[:, 0:XROW], 0.0)
                nc.vector.memset(xs[:, 57 * XROW : XS_TOT], 0.0)
                for ct in range(2):
                    nc.vector.memset(xv[:, 1:57, ct * PADW : ct * PADW + 1], 0.0)
                    nc.vector.memset(
                        xv[:, 1:57, ct * PADW + 57 : ct * PADW + 58], 0.0
                    )

            for t in range(NT):
                gin = gin_pool.tile([128, 512], BF16, name="gin", tag="gin")
                for i, b in enumerate((b0, b1)):
                    nc.sync.dma_start(
                        out=gin[0:TL, i * 256 : (i + 1) * 256],
                        in_=xbt[b, t * TL : (t + 1) * TL, :],
                    )
                sout = sout_pool.tile([128, 512], BF16, name="sout", tag="sout")
                nc.gpsimd.local_scatter(
                    out_ap=sout[0:TL, :],
                    data_ap=gin[0:TL, :],
                    idxs_ap=idxsb[0:TL, t * 512 : (t + 1) * 512],
                    channels=TL,
                    num_elems=512,
                    num_idxs=512,
                )
                for i, b in enumerate((b0, b1)):
                    ps2 = tpsB_pool.tile([128, 2 * TL], BF16, name="ps2", tag="ps2")
                    for ct in range(2):
                        nc.tensor.transpose(
                            ps2[:, ct * TL : ct * TL + TL],
                            sout[0:TL, i * 256 + ct * 128 : i * 256 + ct * 128 + 128],
                            ident[0:TL, 0:TL],
                        )
                    for ct in range(2):
                        q = (2 * t + 1) * XROW + ct * PADW + 1
                        dst = xst[b][:, q : q + 2 * XROW].rearrange(
                            "p (r x) -> p r x", r=2
                        )[:, :, 0:56]
                        src = ps2[:, ct * TL : ct * TL + TL].rearrange(
                            "p (r x) -> p r x", r=2
                        )
                        if ct == 0:
                            nc.scalar.copy(dst, src)
                        else:
                            nc.vector.tensor_copy(dst, src)

            for b in (b0, b1):
                phase2(b, xst[b])

    nc.compile()
    return nc


def _host_prep(x, w, perm):
    import ml_dtypes

    # [B, HW, C] pixel-major bf16 (feeds the scatter without PE fwd transposes)
    xft = np.ascontiguousarray(
        x.reshape(B, C, HW).transpose(0, 2, 1)
    ).astype(ml_dtypes.bfloat16)

    # 1-D winograd weights: U1[j,kh][oc,ic] = sum_kw G[j,kw] w[oc,ic,kh,kw]
    G = np.array([[1, 0, 0], [0.5, 0.5, 0.5], [0.5, -0.5, 0.5], [0, 0, 1]], np.float32)
    U1 = np.einsum("jk,ochk->jhoc", G, w.astype(np.float32))  # [4,3,OC,C]
    u1t = np.empty((48, 128, 128), dtype=ml_dtypes.bfloat16)
    for j in range(4):
        for kh in range(3):
            for ct in range(2):
                for oc in range(2):
                    blk = U1[j, kh][oc * 128 : (oc + 1) * 128, ct * 128 : (ct + 1) * 128]
                    u1t[_tid1(j, kh, ct, oc)] = blk.T.astype(ml_dtypes.bfloat16)

    iperm = np.empty((HW, C), dtype=np.int16)
    np.put_along_axis(
        iperm, perm.astype(np.int64), np.arange(C, dtype=np.int16)[None, :], axis=1
    )
    idxt = np.zeros((128, NT * 512), dtype=np.int16)
    for t in range(NT):
        blk = iperm[t * TL : t * TL + TL, :]
        idxt[0:TL, t * 512 : t * 512 + 256] = blk
        idxt[0:TL, t * 512 + 256 : (t + 1) * 512] = blk + 256

    in_maps = []
    for cidx in range(N_CORES):
        in_maps.append(
            {
                "xbt": np.ascontiguousarray(xft[cidx * B_LOC : (cidx + 1) * B_LOC]),
                "u1": u1t,
                "idxt": idxt,
            }
        )
    return in_maps


def kernel(x, w, perm):
    global LAST_RESULT
    _install_ntff_shim()
    from concourse.bass_utils import run_bass_kernel_spmd

    x = np.asarray(x, dtype=np.float32)
    w = np.asarray(w, dtype=np.float32)
    perm = np.asarray(perm)

    if "nc" not in _STATE:
        _STATE["nc"] = _build_kernel()
    nc = _STATE["nc"]

    in_maps = _host_prep(x, w, perm)
    res = run_bass_kernel_spmd(nc, in_maps, core_ids=list(range(N_CORES)))
    LAST_RESULT = res
    out = np.concatenate(
        [r["out"].reshape(B_LOC, C, H, W) for r in res.results], axis=0
    )
    return out.astype(np.float32)
